# revision 1
# baseline (speedup 1.0000x reference)
"""Causal self-attention block (qkv proj + 16-head causal attention + out_proj
+ c_proj) on 8 trn2 NeuronCores, data-parallel over the batch (B=8: one batch
element per core).

Layout strategy (per core, batch element b):
  - Activations are kept feature-major [feature, token] on chip so every
    linear layer is a plain   out = W_T.T @ act   matmul chain with the
    (host-pre-transposed) weight as the stationary operand. No on-device
    transposes at all.
  - Attention computes transposed scores  sT[tk, tq] = k_h.T q_h  per head
    pair (row-tiled K=64 matmuls run concurrently on the PE), exp with no
    max-subtraction (scores here are bounded by a few units), causal mask
    accumulated into the scores psum by a bf16 identity-matmul, and the AV
    product consumes sT directly with token-major V tiles as the stationary
    operand. A fused ones-row in the V operand (M=65) yields the softmax
    denominator for free; batched reciprocals and K=16 indicator matmuls
    broadcast 1/denom across partitions for the normalization.
  - All matmuls run in float32r (TF32-like, ~1e-4 rel precision, 4x the
    throughput of fp32 on the PE).
"""

import sys

if "/opt/trn_rl_repo" not in sys.path:
    sys.path.insert(0, "/opt/trn_rl_repo")

import ml_dtypes
import numpy as np

import concourse.bass as bass  # noqa: F401  (bass types used via tile/bacc)
import concourse.tile as tile
from concourse import bacc, mybir
from concourse.bass_utils import run_bass_kernel_spmd

B, T, E, H = 8, 1024, 1024, 16
DH = E // H          # 64
JQK = 2 * E          # q+k fused feature dim (2048)
F32 = mybir.dt.float32
F32R = mybir.dt.float32r
BF16 = mybir.dt.bfloat16
Act = mybir.ActivationFunctionType

TRACE = False        # test harness flips this for profiled runs
PHASE_LIMIT = 4      # debug: 1=qk proj, 2=+v, 3=+attention, 4=full
_CACHE = {}


def _emit(nc, tc, aps):
    (xT, wqkT, wvT, bqk, bvrow, woutT, bout, wcT, bc, mask01, ones,
     onesbf, outT) = aps
    ET = E // 128     # 8  e-tiles (contraction)
    TT = T // 128     # 8  token tiles
    NT = T // 512     # 2  512-wide token column chunks

    # The kernel is emitted as one software-pipelined stream so the in-order
    # PE queue always has dense matmul work while ACT paces the attention
    # exps:
    #   S1: qk features for pairs 0-3 (j-groups 0,2) + V heads 0-7
    #   S2: attention (c0+c1, pairs 0-3) interleaved with qk j-groups 1,3 +
    #       V heads 8-15
    #   S3: attention (c0, pairs 4-7); normalize c0; attention (c1, pairs
    #       4-7) interleaved with out_proj on token-chunk 0
    #   S4: normalize c1; out_proj chunk 1; c_proj
    # Pool stack (LIFO): den/y/qk/v persist; x/wqk and the S2 attention pools
    # release at the S2/S3 boundary where w3/z and the S3 attention pools
    # open in their space.
    consts = tc.alloc_tile_pool(name="consts", bufs=1)
    onesb = consts.tile([128, 128], F32R, tag="onesb")
    mask01b = consts.tile([128, 128], BF16, tag="mask01b")
    bqkb = consts.tile([128, JQK // 128], F32, tag="bqkb")
    bvb = consts.tile([1, E], F32R, tag="bvb")
    boutb = consts.tile([128, E // 128], F32, tag="boutb")
    bcb = consts.tile([128, E // 128], F32, tag="bcb")
    nc.sync.dma_start(out=onesb, in_=ones)
    nc.sync.dma_start(out=mask01b, in_=mask01)
    nc.sync.dma_start(out=bqkb, in_=bqk)
    nc.sync.dma_start(out=bvb[0:1, :], in_=bvrow)
    nc.sync.dma_start(out=boutb, in_=bout)
    nc.sync.dma_start(out=bcb, in_=bc)

    psum = tc.alloc_tile_pool(name="psum", bufs=1, space="PSUM")
    p_dram = tc.alloc_tile_pool(name="p_dram", bufs=1, space="DRAM")
    p_den = tc.alloc_tile_pool(name="p_den", bufs=1)
    p_y = tc.alloc_tile_pool(name="p_y", bufs=1)
    p_qk = tc.alloc_tile_pool(name="p_qk", bufs=1)
    p_v = tc.alloc_tile_pool(name="p_v", bufs=1)
    p_x = tc.alloc_tile_pool(name="p_x", bufs=1)
    p_wqk = tc.alloc_tile_pool(name="p_wqk", bufs=16)
    dden = p_dram.tile([64, 512], F32, tag="dden")
    denall = p_den.tile([64, 512], F32, tag="denall")
    recall = p_den.tile([64, 512], F32, tag="recall")
    yt = p_y.tile([128, ET, T], F32R)
    qkt = p_qk.tile([128, JQK // 128, T], F32R)
    vt = p_v.tile([128, TT, H, DH + 1], BF16)
    xt = p_x.tile([128, ET, T], F32R)

    def mm_psum(tag):
        return psum.tile([128, 512], F32, tag=tag, bufs=2, name="ps_" + tag)

    # ---- dense generators: qkv projection ---------------------------------
    def qk_gen(jg):
        """qkT[j, t] = Wqk x^T + bqk for the 512-wide feature group jg."""
        wtiles = []
        for et in range(ET):
            if jg == 0:                    # interleave x loads with group 0
                nc.sync.dma_start(out=xt[:, et, :],
                                  in_=xT[et * 128:(et + 1) * 128, :])
            wt = p_wqk.tile([128, 512], F32R, tag="wqk", name="wt")
            nc.sync.dma_start(out=wt, in_=wqkT[et * 128:(et + 1) * 128,
                                              jg * 512:(jg + 1) * 512])
            wtiles.append(wt)
        for js in range(4):
            jt = jg * 4 + js
            for th in range(NT):
                ps = mm_psum("mm")
                for et in range(ET):
                    nc.tensor.matmul(
                        ps,
                        wtiles[et][:, js * 128:(js + 1) * 128],
                        xt[:, et, th * 512:(th + 1) * 512],
                        start=(et == 0), stop=(et == ET - 1))
                    yield
                nc.scalar.activation(
                    out=qkt[:, jt, th * 512:(th + 1) * 512], in_=ps,
                    func=Act.Identity, bias=bqkb[:, jt:jt + 1], scale=1.0)

    def vb_gen(jh):
        """v[t, h, d] token-major for heads 8*jh..8*jh+7 (+bias via ones-row
        matmul), with a bf16 ones column at d=64 for the fused denominator."""
        if jh == 0:
            for tt in range(TT):
                nc.sync.dma_start(out=vt[:, tt, :, DH], in_=onesbf)
        wvtiles = []
        for et in range(ET):
            wt = p_wqk.tile([128, 512], F32R, tag="wqk", name="wt")
            nc.sync.dma_start(out=wt, in_=wvT[et * 128:(et + 1) * 128,
                                             jh * 512:(jh + 1) * 512])
            wvtiles.append(wt)
        for tt in range(TT):
            ps = mm_psum("mm")
            for et in range(ET):
                nc.tensor.matmul(
                    ps,
                    xt[:, et, tt * 128:(tt + 1) * 128],
                    wvtiles[et],
                    start=(et == 0), stop=False)
                yield
            nc.tensor.matmul(
                ps, onesb[0:1, 0:128], bvb[0:1, jh * 512:(jh + 1) * 512],
                start=False, stop=True)
            yield
            nc.vector.tensor_copy(
                out=vt[:, tt, jh * 8:(jh + 1) * 8, 0:DH],
                in_=ps.rearrange("p (h d) -> p h d", d=DH))

    # ---- attention generator (yields once per tk-iteration) ---------------
    LAG = 3

    def att_gen(c, a, p_esc, p_nrm):
        cs = c * 512
        last_it = 4 * c + 3
        qj = a                             # q tile of the pair
        kj = (JQK // 2) // 128 + a         # k tile of the pair
        avps = [psum.tile([128, 512], F32, tag=f"av{p}", bufs=1,
                          name=f"avp{p}") for p in range(2)]
        pend = []

        def emit_av(it, sub, clen, esc):
            for p in range(2):
                nc.tensor.matmul(
                    avps[p][0:DH + 1, sub:sub + clen],
                    vt[:, it, 2 * a + p, :],
                    esc[:, p, :clen],
                    start=(it == 0), stop=(it == last_it),
                    skip_group_check=True)

        for it in range(last_it + 1):
            n0 = it * 128
            lo = max(n0, cs)
            sub = lo - cs
            clen = 512 - sub
            scp = psum.tile([128, 2, 512], F32, tag="sc", bufs=2, name="scp")
            for p in range(2):             # paired heads: row-tiled matmuls
                pb = p * 64
                nc.tensor.matmul(
                    scp[:, p, :clen],
                    qkt[pb:pb + 64, kj, n0:n0 + 128],
                    qkt[pb:pb + 64, qj, lo:lo + clen],
                    start=True, stop=True)
            esc = p_esc.tile([128, 2, 512], BF16, tag="esc", name="esc")
            nc.scalar.activation(out=esc[:, :, :clen], in_=scp[:, :, :clen],
                                 func=Act.Exp, scale=1.0 / 8.0)
            if n0 >= cs:                   # diagonal block: causal mask,
                nc.vector.tensor_mul(      # off the PE chain thanks to LAG
                    esc[:, :, 0:128], esc[:, :, 0:128],
                    mask01b[:, None, :].broadcast_to([128, 2, 128]))
            pend.append((it, sub, clen, esc))
            if len(pend) > LAG:
                emit_av(*pend.pop(0))
            yield
        for args in pend:
            emit_av(*args)
        for p in range(2):                 # drain unnormalized y + denom row
            h = 2 * a + p
            nc.vector.tensor_copy(out=yt[p * 64:p * 64 + 64, qj,
                                         cs:cs + 512],
                                  in_=avps[p][0:DH, :])
            # engines can only address partition bases that are multiples of
            # 32, so stage the denominator row at partition 64 and DMA-
            # scatter it (partition-agnostic) into denall's row.
            stg = p_nrm.tile([128, 512], F32, tag="stg", bufs=2, name="stg")
            nc.vector.tensor_copy(out=stg[64:65, :],
                                  in_=avps[p][DH:DH + 1, :])
            nc.sync.dma_start(out=denall[32 * c + h:32 * c + h + 1, :],
                              in_=stg[64:65, :])

    def norm_unit(c, p_nrm):
        """Batched 1/denom for chunk c, bounced through DRAM and partition-
        broadcast back; normalizes y in place on DVE."""
        r0 = 32 * c
        cs = c * 512
        with nc.allow_low_precision(reason="fp32 reciprocal feeding an f32r "
                                    "multiply; well inside tolerance"):
            nc.vector.reciprocal(out=recall[r0:r0 + 16, :],
                                 in_=denall[r0:r0 + 16, :])
        nc.sync.dma_start(out=dden[r0:r0 + 16, :], in_=recall[r0:r0 + 16, :])
        for a in range(H // 2):
            rb = p_nrm.tile([128, 512], F32, tag="rb", bufs=2, name="rb")
            for p in range(2):
                row = dden[r0 + 2 * a + p:r0 + 2 * a + p + 1, :]
                src = bass.AP(tensor=row.tensor, offset=row.offset,
                              ap=[[0, 64]] + list(row.ap)[1:])
                nc.sync.dma_start(out=rb[p * 64:(p + 1) * 64, :], in_=src)
            nc.vector.tensor_mul(yt[:, a, cs:cs + 512],
                                 yt[:, a, cs:cs + 512], rb)

    # ---- drivers ----------------------------------------------------------
    def run_dense(dense, n=None):
        steps = 0
        while dense and (n is None or steps < n):
            try:
                next(dense[0])
                steps += 1
            except StopIteration:
                dense.pop(0)
        return steps

    def drive(att_units, dense, ratio=5):
        att_units = list(att_units)
        while att_units:
            try:
                next(att_units[0])
            except StopIteration:
                att_units.pop(0)
                continue
            run_dense(dense, ratio)
        run_dense(dense)

    # S1: dense deps for attention pairs 0-3
    dense1 = [qk_gen(0), qk_gen(2)] + ([vb_gen(0)] if PHASE_LIMIT >= 2 else [])
    run_dense(dense1)

    # S2: attention pairs 0-3 (both chunks) over the remaining qkv work
    p_esc1 = tc.alloc_tile_pool(name="p_esc1", bufs=4)
    p_nrm1 = tc.alloc_tile_pool(name="p_nrm1", bufs=1)
    dense2 = [qk_gen(1), qk_gen(3)] + ([vb_gen(1)] if PHASE_LIMIT >= 2 else [])
    att2 = [att_gen(c, a, p_esc1, p_nrm1)
            for a in range(4) for c in range(NT)] if PHASE_LIMIT >= 3 else []
    drive(att2, dense2)
    p_nrm1.release()
    p_esc1.release()
    p_wqk.release()
    p_x.release()

    # S3: remaining attention; out_proj weight prefetch + chunk-0 out_proj
    p_w3 = tc.alloc_tile_pool(name="p_w3", bufs=16)
    p_z = tc.alloc_tile_pool(name="p_z", bufs=1)
    p_esc2 = tc.alloc_tile_pool(name="p_esc2", bufs=4)
    p_nrm2 = tc.alloc_tile_pool(name="p_nrm2", bufs=1)
    zt = p_z.tile([128, ET, T], F32R)
    wout_tiles = []
    if PHASE_LIMIT >= 4:
        for og in range(2):
            for et in range(ET):
                wt = p_w3.tile([128, 512], F32R, tag="w3", name="wt3")
                nc.sync.dma_start(
                    out=wt, in_=woutT[et * 128:(et + 1) * 128,
                                      og * 512:(og + 1) * 512])
                wout_tiles.append(wt)

    def oproj_gen(th):
        for og in range(2):
            for os_ in range(4):
                ot = og * 4 + os_
                ps = mm_psum("mm")
                for et in range(ET):
                    nc.tensor.matmul(
                        ps,
                        wout_tiles[og * ET + et][:, os_ * 128:(os_ + 1) * 128],
                        yt[:, et, th * 512:(th + 1) * 512],
                        start=(et == 0), stop=(et == ET - 1))
                    yield
                nc.scalar.activation(
                    out=zt[:, ot, th * 512:(th + 1) * 512], in_=ps,
                    func=Act.Identity, bias=boutb[:, ot:ot + 1], scale=1.0)

    def cproj_gen(wts, og, th):
        for os_ in range(4):
            ot = og * 4 + os_
            ps = mm_psum("mm")
            for et in range(ET):
                nc.tensor.matmul(
                    ps,
                    wts[et][:, os_ * 128:(os_ + 1) * 128],
                    zt[:, et, th * 512:(th + 1) * 512],
                    start=(et == 0), stop=(et == ET - 1))
                yield
            ob = p_out.tile([128, 512], F32, tag="ob", name="ob")
            nc.scalar.activation(out=ob, in_=ps, func=Act.Identity,
                                 bias=bcb[:, ot:ot + 1], scale=1.0)
            nc.sync.dma_start(
                out=outT[ot * 128:(ot + 1) * 128, th * 512:(th + 1) * 512],
                in_=ob)

    if PHASE_LIMIT >= 3:
        drive([att_gen(0, a, p_esc2, p_nrm2) for a in range(4, 8)], [])
        norm_unit(0, p_nrm2)
        drive([att_gen(1, a, p_esc2, p_nrm2) for a in range(4, 8)],
              [oproj_gen(0)] if PHASE_LIMIT >= 4 else [])
        norm_unit(1, p_nrm2)
    p_nrm2.release()
    p_esc2.release()

    # S4: c_proj — its own weight pool so zt-chunk-0 c_proj can run ahead of
    # oproj(1) (which waits on the chunk-1 normalization chain).
    p_wc = tc.alloc_tile_pool(name="p_wc", bufs=9)
    p_out = tc.alloc_tile_pool(name="p_out", bufs=2)

    def load_wc(og):
        wts = []
        for et in range(ET):
            wt = p_wc.tile([128, 512], F32R, tag="wc", name="wtc")
            nc.sync.dma_start(out=wt, in_=wcT[et * 128:(et + 1) * 128,
                                             og * 512:(og + 1) * 512])
            wts.append(wt)
        return wts

    if PHASE_LIMIT >= 4:
        wc0 = load_wc(0)
        run_dense([cproj_gen(wc0, 0, 0)])
        run_dense([oproj_gen(1)])
        wc1 = load_wc(1)
        run_dense([cproj_gen(wc0, 0, 1)])
        run_dense([cproj_gen(wc1, 1, 0), cproj_gen(wc1, 1, 1)])
    p_out.release()
    p_wc.release()
    p_z.release()
    p_w3.release()
    p_v.release()
    p_qk.release()
    p_y.release()
    p_den.release()
    p_dram.release()
    psum.release()
    consts.release()


def _build():
    if "nc" in _CACHE:
        return _CACHE["nc"]
    nc = bacc.Bacc("TRN2", target_bir_lowering=False, debug=False,
                   enable_asserts=True, num_devices=8)
    d = nc.dram_tensor
    aps = [
        d("xT", [E, T], F32R, kind="ExternalInput").ap(),
        d("wqkT", [E, JQK], F32R, kind="ExternalInput").ap(),
        d("wvT", [E, E], F32R, kind="ExternalInput").ap(),
        d("bqk", [128, JQK // 128], F32, kind="ExternalInput").ap(),
        d("bvrow", [1, E], F32R, kind="ExternalInput").ap(),
        d("woutT", [E, E], F32R, kind="ExternalInput").ap(),
        d("bout", [128, E // 128], F32, kind="ExternalInput").ap(),
        d("wcT", [E, E], F32R, kind="ExternalInput").ap(),
        d("bc", [128, E // 128], F32, kind="ExternalInput").ap(),
        d("mask01", [128, 128], BF16, kind="ExternalInput").ap(),
        d("ones", [128, 128], F32R, kind="ExternalInput").ap(),
        d("onesbf", [128, H], BF16, kind="ExternalInput").ap(),
        d("outT", [E, T], F32, kind="ExternalOutput").ap(),
    ]
    with tile.TileContext(nc) as tc:
        _emit(nc, tc, aps)
    nc.compile()
    _CACHE["nc"] = nc
    return nc


def _host_inputs(x, in_proj_w, in_proj_b, out_proj_w, out_proj_b,
                 c_proj_w, c_proj_b):
    f = np.float32
    x = np.ascontiguousarray(np.asarray(x, f))
    in_proj_w = np.asarray(in_proj_w, f)
    in_proj_b = np.asarray(in_proj_b, f)
    shared = {
        "wqkT": np.ascontiguousarray(in_proj_w[:JQK].T),
        "wvT": np.ascontiguousarray(in_proj_w[JQK:].T),
        "bqk": np.ascontiguousarray(in_proj_b[:JQK].reshape(JQK // 128, 128).T),
        "bvrow": np.ascontiguousarray(in_proj_b[JQK:].reshape(1, E)),
        "woutT": np.ascontiguousarray(np.asarray(out_proj_w, f).T),
        "bout": np.ascontiguousarray(
            np.asarray(out_proj_b, f).reshape(E // 128, 128).T),
        "wcT": np.ascontiguousarray(np.asarray(c_proj_w, f).T),
        "bc": np.ascontiguousarray(
            np.asarray(c_proj_b, f).reshape(E // 128, 128).T),
        "mask01": np.where(np.arange(128)[None, :] >= np.arange(128)[:, None],
                           f(1.0), f(0.0)).astype(ml_dtypes.bfloat16),
        "ones": np.ones((128, 128), f),
        "onesbf": np.ones((128, H), ml_dtypes.bfloat16),
    }
    return [{**shared, "xT": np.ascontiguousarray(x[b].T)} for b in range(B)]


def kernel(x, in_proj_w, in_proj_b, out_proj_w, out_proj_b, c_proj_w,
           c_proj_b):
    nc = _build()
    in_maps = _host_inputs(x, in_proj_w, in_proj_b, out_proj_w, out_proj_b,
                           c_proj_w, c_proj_b)
    res = run_bass_kernel_spmd(nc, in_maps, core_ids=list(range(B)),
                               trace=TRACE)
    _CACHE["last_result"] = res
    out = np.stack([res.results[b]["outT"].T for b in range(B)])
    return np.ascontiguousarray(out, dtype=np.float32)



# revision 14
# speedup vs baseline: 1.1836x; 1.1836x over previous
"""Causal self-attention block (qkv proj + 16-head causal attention + out_proj
+ c_proj) on 8 trn2 NeuronCores, data-parallel over the batch (B=8: one batch
element per core).

Layout strategy (per core, batch element b):
  - All matmuls in bf16 (1 cycle/row on the PE at ANY moving size, unlike
    fp32r's 4x penalty below 256 rows); fp32 PSUM accumulation. End-to-end
    numpy sim of this quantization gives rel_err ~3.6e-3 vs the 2e-2 gate.
  - Activations feature-major [feature, token] so every linear layer is a
    plain  out = W_T.T @ act  matmul chain with the host-pre-transposed
    weight stationary. No on-device transposes.
  - Attention runs in NT=4 query chunks of W=256 so the tail (last chunk's
    normalize -> out_proj -> c_proj) is short; out_proj/c_proj of chunk c
    overlap attention of chunks > c.
  - Attention computes transposed scores sT[tk, tq] = k_h.T q_h per head
    pair (row-tiled K=64 matmuls), exp on ACT (no max-subtraction; scores
    are bounded), causal mask as a bf16 multiply on DVE, and AV consumes
    sT with token-major V stationary (fused ones-column yields the softmax
    denominator for free).
  - Softmax normalization: per-chunk denominators are DMA-gathered into a
    [16, W] tile, batch-reciprocated on DVE, then broadcast across the 128
    y-partitions by a K=16 indicator matmul into PSUM (no DRAM bounce, no
    per-head broadcast DMAs) and applied by DVE multiplies.
  - PSUM->SBUF drains: qk / out_proj bias-adds on ACT (Identity with
    per-partition bias), v / c_proj bias-adds on DVE (GPSIMD cannot read
    PSUM on TRN2).
"""

import sys

if "/opt/trn_rl_repo" not in sys.path:
    sys.path.insert(0, "/opt/trn_rl_repo")

import ml_dtypes
import numpy as np

import concourse.bass as bass  # noqa: F401
import concourse.tile as tile
from concourse import bacc, mybir
from concourse.bass_utils import run_bass_kernel_spmd

B, T, E, H = 8, 1024, 1024, 16
DH = E // H          # 64
JQK = 2 * E          # q+k fused feature dim (2048)
NT = 4               # attention query chunks
W = T // NT          # 256
ET = E // 128        # 8
TT = T // 128        # 8
KJ0 = JQK // 128 // 2  # 8: first k feature-tile index in qkt
F32 = mybir.dt.float32
BF16 = mybir.dt.bfloat16
Act = mybir.ActivationFunctionType

TRACE = False        # test harness flips this for profiled runs
PHASE_LIMIT = 4      # debug: 1=qk proj, 2=+v, 3=+attention, 4=full
LAG = 3              # exp->AV lag (its) so the mask multiply is off-chain
RATIO_S2 = 3         # dense matmuls interleaved per attention yield, S2
RATIO_S3 = 5         # .. S3 (dense is N=256 there, attention needs ACT time)
_CACHE = {}


def _emit(nc, tc, aps):
    (xT, wqkT, wvT, bqk, bvrow, woutT, bout, wcT, bc, mask01, onesbf, ind,
     outT) = aps

    consts = tc.alloc_tile_pool(name="consts", bufs=1)
    mask01b = consts.tile([128, 128], BF16, tag="mask01b")
    bqkb = consts.tile([128, JQK // 128], F32, tag="bqkb")
    bvb = consts.tile([128, E], BF16, tag="bvb")
    boutb = consts.tile([128, E // 128], F32, tag="boutb")
    bcb = consts.tile([128, E // 128], F32, tag="bcb")
    onesc = consts.tile([128, H], BF16, tag="onesc")
    indb = consts.tile([16, H // 2, 128], BF16, tag="indb")
    nc.sync.dma_start(out=mask01b, in_=mask01)
    nc.sync.dma_start(out=bqkb, in_=bqk)
    # v-bias broadcast [1, E] -> [128, E] via zero-stride partition read
    bsrc = bass.AP(tensor=bvrow.tensor, offset=bvrow.offset,
                   ap=[[0, 128]] + list(bvrow.ap)[1:])
    nc.sync.dma_start(out=bvb, in_=bsrc)
    nc.sync.dma_start(out=boutb, in_=bout)
    nc.sync.dma_start(out=bcb, in_=bc)
    nc.sync.dma_start(out=onesc, in_=onesbf)
    nc.sync.dma_start(out=indb, in_=ind)

    # attention PSUM: scp padded so each head-pair's accumulation lives in
    # its own 2KB bank; av0/av1 likewise separate banks (4+1+1 = 6 banks).
    # The dense-GEMM psum pools (2 banks each) swap at the S2/S3 boundary.
    psum = tc.alloc_tile_pool(name="psum", bufs=1, space="PSUM")
    p_den = tc.alloc_tile_pool(name="p_den", bufs=1)
    p_nrm = tc.alloc_tile_pool(name="p_nrm", bufs=1)
    p_y = tc.alloc_tile_pool(name="p_y", bufs=1)
    p_qk = tc.alloc_tile_pool(name="p_qk", bufs=1)
    p_v = tc.alloc_tile_pool(name="p_v", bufs=1)
    p_z = tc.alloc_tile_pool(name="p_z", bufs=1)
    p_w34 = tc.alloc_tile_pool(name="p_w34", bufs=32)
    p_x = tc.alloc_tile_pool(name="p_x", bufs=1)
    p_wqk = tc.alloc_tile_pool(name="p_wqk", bufs=16)
    psum_mm = tc.alloc_tile_pool(name="psum_mm", bufs=1, space="PSUM")

    denc = p_den.tile([16, NT, W], F32, tag="denc")
    recc = p_den.tile([16, NT, W], BF16, tag="recc")
    yt = p_y.tile([128, ET, T], BF16)
    qkt = p_qk.tile([128, JQK // 128, T], BF16)
    vt = p_v.tile([128, TT, H, DH + 1], BF16)
    zt = p_z.tile([128, ET, T], BF16)
    xt = p_x.tile([128, ET, T], BF16)

    # ---- dense generators: qkv projection ---------------------------------
    def qk_gen(jg):
        """qkT[j, t] = Wqk x^T + bqk for the 512-wide feature group jg."""
        wtiles = []
        for et in range(ET):
            if jg == 0:                    # interleave x loads with group 0
                nc.sync.dma_start(out=xt[:, et, :],
                                  in_=xT[et * 128:(et + 1) * 128, :])
            wt = p_wqk.tile([128, 512], BF16, tag="wqk", name="wt")
            nc.sync.dma_start(out=wt, in_=wqkT[et * 128:(et + 1) * 128,
                                              jg * 512:(jg + 1) * 512])
            wtiles.append(wt)
        for js in range(4):
            jt = jg * 4 + js
            for th in range(2):
                ps = psum_mm.tile([128, 512], F32, tag="mm", bufs=2, name="pmm")
                for et in range(ET):
                    nc.tensor.matmul(
                        ps,
                        wtiles[et][:, js * 128:(js + 1) * 128],
                        xt[:, et, th * 512:(th + 1) * 512],
                        start=(et == 0), stop=(et == ET - 1))
                    yield
                nc.scalar.activation(
                    out=qkt[:, jt, th * 512:(th + 1) * 512], in_=ps,
                    func=Act.Identity, bias=bqkb[:, jt:jt + 1], scale=1.0)

    def vb_gen(jh):
        """v[t, h, d] token-major for heads 8*jh..8*jh+7 (+bias on Pool),
        with a bf16 ones column at d=64 for the fused denominator."""
        if jh == 0:
            for tt in range(TT):
                nc.sync.dma_start(out=vt[:, tt, :, DH], in_=onesc)
        wvtiles = []
        for et in range(ET):
            wt = p_wqk.tile([128, 512], BF16, tag="wqk", name="wt")
            nc.sync.dma_start(out=wt, in_=wvT[et * 128:(et + 1) * 128,
                                             jh * 512:(jh + 1) * 512])
            wvtiles.append(wt)
        bvv = bvb.rearrange("p (h d) -> p h d", d=DH)
        for tt in range(TT):
            ps = psum_mm.tile([128, 512], F32, tag="mm", bufs=2, name="pmm")
            for et in range(ET):
                nc.tensor.matmul(
                    ps,
                    xt[:, et, tt * 128:(tt + 1) * 128],
                    wvtiles[et],
                    start=(et == 0), stop=(et == ET - 1))
                yield
            nc.vector.tensor_add(
                out=vt[:, tt, jh * 8:(jh + 1) * 8, 0:DH],
                in0=ps.rearrange("p (h d) -> p h d", d=DH),
                in1=bvv[:, jh * 8:(jh + 1) * 8, :])

    def w34_loader():
        """Prefetch out_proj + c_proj weights during S2 (long before use)."""
        if PHASE_LIMIT < 4:
            return
        for dst, src in ((wout_tiles, woutT), (wc_tiles, wcT)):
            for og in range(2):
                for et in range(ET):
                    wt = p_w34.tile([128, 512], BF16, tag="w34", name="wt3")
                    nc.sync.dma_start(
                        out=wt, in_=src[et * 128:(et + 1) * 128,
                                        og * 512:(og + 1) * 512])
                    dst.append(wt)
                    yield

    # ---- attention generator (yields once per tk-iteration) ---------------
    def att_gen(c, a, p_esc):
        cs = c * W
        last_it = 2 * c + 1
        avps = [psum.tile([128, W], F32, tag=f"av{p}", bufs=1,
                          name=f"avp{p}") for p in range(2)]
        pend = []

        def emit_av(it, sub, clen, esc):
            for p in range(2):
                nc.tensor.matmul(
                    avps[p][0:DH + 1, sub:sub + clen],
                    vt[:, it, 2 * a + p, :],
                    esc[:, p, :clen],
                    start=(it == 0), stop=(it == last_it),
                    skip_group_check=True)

        for it in range(last_it + 1):
            n0 = it * 128
            lo = max(n0, cs)
            sub = lo - cs
            clen = W - sub
            scp = psum.tile([128, 2, W], F32, tag="scp", bufs=2, name="scp",
                            padded_shape=[128, 2, 512])
            for p in range(2):             # paired heads: row-tiled matmuls
                pb = p * 64
                nc.tensor.matmul(
                    scp[:, p, :clen],
                    qkt[pb:pb + 64, KJ0 + a, n0:n0 + 128],
                    qkt[pb:pb + 64, a, lo:lo + clen],
                    start=True, stop=True)
            esc = p_esc.tile([128, 2, W], BF16, tag="esc", name="esc")
            nc.scalar.activation(out=esc[:, :, :clen], in_=scp[:, :, :clen],
                                 func=Act.Exp, scale=1.0 / 8.0)
            if n0 >= cs:                   # diagonal block: causal mask
                nc.vector.tensor_mul(
                    esc[:, :, 0:128], esc[:, :, 0:128],
                    mask01b[:, None, :].broadcast_to([128, 2, 128]))
            pend.append((it, sub, clen, esc))
            if len(pend) > LAG:
                emit_av(*pend.pop(0))
            yield
        for args in pend:
            emit_av(*args)
        # stage the denominator rows at partition 64 (engines address
        # partition bases in multiples of 32) and DMA-scatter (partition-
        # agnostic) into this chunk's [16, W] denominator tile.
        stg = p_nrm.tile([128, 2, W], F32, tag="stg", bufs=2, name="stg")
        for p in range(2):                 # drain unnormalized y + denom row
            nc.vector.tensor_copy(out=yt[p * 64:p * 64 + 64, a, cs:cs + W],
                                  in_=avps[p][0:DH, :])
            nc.vector.tensor_copy(out=stg[64:65, p, :],
                                  in_=avps[p][DH:DH + 1, :])
        nc.sync.dma_start(out=denc[2 * a:2 * a + 2, c, :],
                          in_=stg[64:65, :, :])

    def norm_gen(c):
        """1/denom for chunk c (batched DVE reciprocal), broadcast across
        partitions by K=16 indicator matmuls, applied in place on DVE."""
        cs = c * W
        with nc.allow_low_precision(reason="fp32 reciprocal feeding a bf16 "
                                    "multiply; well inside tolerance"):
            nc.vector.reciprocal(out=recc[:, c, :], in_=denc[:, c, :])
        yield
        for a in range(H // 2):
            rb = psum_mo.tile([128, W], F32, tag="mo", bufs=2, name="rb")
            nc.tensor.matmul(rb, indb[:, a, :], recc[:, c, :],
                             start=True, stop=True)
            yield
            nc.vector.tensor_mul(yt[:, a, cs:cs + W], yt[:, a, cs:cs + W],
                                 rb)
            yield

    def oproj_gen(c):
        cs = c * W
        for ot in range(ET):
            og, os_ = divmod(ot, 4)
            ps = psum_mo.tile([128, W], F32, tag="mo", bufs=2, name="po")
            for et in range(ET):
                nc.tensor.matmul(
                    ps,
                    wout_tiles[og * ET + et][:, os_ * 128:(os_ + 1) * 128],
                    yt[:, et, cs:cs + W],
                    start=(et == 0), stop=(et == ET - 1))
                yield
            nc.scalar.activation(out=zt[:, ot, cs:cs + W], in_=ps,
                                 func=Act.Identity, bias=boutb[:, ot:ot + 1],
                                 scale=1.0)

    def cproj_gen(c):
        cs = c * W
        for ot in range(ET):
            og, os_ = divmod(ot, 4)
            ps = psum_mo.tile([128, W], F32, tag="mo", bufs=2, name="pc")
            for et in range(ET):
                nc.tensor.matmul(
                    ps,
                    wc_tiles[og * ET + et][:, os_ * 128:(os_ + 1) * 128],
                    zt[:, et, cs:cs + W],
                    start=(et == 0), stop=(et == ET - 1))
                yield
            ob = p_out.tile([128, W], F32, tag="ob", bufs=3, name="ob")
            nc.vector.tensor_scalar_add(out=ob, in0=ps,
                                        scalar1=bcb[:, ot:ot + 1])
            nc.sync.dma_start(out=outT[ot * 128:(ot + 1) * 128, cs:cs + W],
                              in_=ob)

    # ---- drivers ----------------------------------------------------------
    def run_dense(dense, n=None):
        steps = 0
        while dense and (n is None or steps < n):
            try:
                next(dense[0])
                steps += 1
            except StopIteration:
                dense.pop(0)
        return steps

    def drive(att_units, dense, ratio):
        att_units = list(att_units)
        while att_units:
            try:
                next(att_units[0])
            except StopIteration:
                att_units.pop(0)
                continue
            run_dense(dense, ratio)
        run_dense(dense)

    wout_tiles = []
    wc_tiles = []

    # S1: dense-only warmup — deps for attention pairs 0-3
    dense1 = [qk_gen(0), qk_gen(2)] + ([vb_gen(0)] if PHASE_LIMIT >= 2 else [])
    run_dense(dense1)

    # S2: attention pairs 0-3 (all chunks) over the remaining qkv work,
    # with out/c_proj weight prefetch at the back of the DMA queue
    p_esc1 = tc.alloc_tile_pool(name="p_esc1", bufs=6)
    dense2 = [qk_gen(1), qk_gen(3)] + ([vb_gen(1)] if PHASE_LIMIT >= 2 else [])
    dense2.append(w34_loader())
    att2 = [att_gen(c, a, p_esc1)
            for a in range(4) for c in range(NT)] if PHASE_LIMIT >= 3 else []
    drive(att2, dense2, RATIO_S2)
    p_esc1.release()
    psum_mm.release()
    p_wqk.release()
    p_x.release()

    # S3/S4: attention pairs 4-7 chunk by chunk; after each chunk completes,
    # its normalize + out_proj + c_proj join the dense stream
    psum_mo = tc.alloc_tile_pool(name="psum_mo", bufs=1, space="PSUM")
    p_esc2 = tc.alloc_tile_pool(name="p_esc2", bufs=6)
    p_out = tc.alloc_tile_pool(name="p_out", bufs=3)
    dense3 = []
    if PHASE_LIMIT >= 3:
        for c in range(NT):
            drive([att_gen(c, a, p_esc2) for a in range(4, 8)],
                  dense3, RATIO_S3)
            if PHASE_LIMIT >= 4:
                dense3.append(norm_gen(c))
                dense3.append(oproj_gen(c))
                dense3.append(cproj_gen(c))
    run_dense(dense3)

    p_out.release()
    p_esc2.release()
    psum_mo.release()
    p_w34.release()
    p_z.release()
    p_v.release()
    p_qk.release()
    p_y.release()
    p_nrm.release()
    p_den.release()
    psum.release()
    consts.release()


def _build():
    if "nc" in _CACHE:
        return _CACHE["nc"]
    nc = bacc.Bacc("TRN2", target_bir_lowering=False, debug=False,
                   enable_asserts=True, num_devices=8)
    d = nc.dram_tensor
    aps = [
        d("xT", [E, T], BF16, kind="ExternalInput").ap(),
        d("wqkT", [E, JQK], BF16, kind="ExternalInput").ap(),
        d("wvT", [E, E], BF16, kind="ExternalInput").ap(),
        d("bqk", [128, JQK // 128], F32, kind="ExternalInput").ap(),
        d("bvrow", [1, E], BF16, kind="ExternalInput").ap(),
        d("woutT", [E, E], BF16, kind="ExternalInput").ap(),
        d("bout", [128, E // 128], F32, kind="ExternalInput").ap(),
        d("wcT", [E, E], BF16, kind="ExternalInput").ap(),
        d("bc", [128, E // 128], F32, kind="ExternalInput").ap(),
        d("mask01", [128, 128], BF16, kind="ExternalInput").ap(),
        d("onesbf", [128, H], BF16, kind="ExternalInput").ap(),
        d("ind", [16, (H // 2) * 128], BF16, kind="ExternalInput").ap(),
        d("outT", [E, T], F32, kind="ExternalOutput").ap(),
    ]
    with tile.TileContext(nc) as tc:
        _emit(nc, tc, aps)
    nc.compile()
    _CACHE["nc"] = nc
    return nc


def _host_inputs(x, in_proj_w, in_proj_b, out_proj_w, out_proj_b,
                 c_proj_w, c_proj_b):
    f = np.float32
    bf = ml_dtypes.bfloat16
    x = np.asarray(x, f)
    in_proj_w = np.asarray(in_proj_w, f)
    in_proj_b = np.asarray(in_proj_b, f)
    ind = np.zeros((16, H // 2, 128), f)
    for a in range(H // 2):
        ind[2 * a, a, 0:64] = 1.0
        ind[2 * a + 1, a, 64:128] = 1.0
    shared = {
        "wqkT": np.ascontiguousarray(in_proj_w[:JQK].T).astype(bf),
        "wvT": np.ascontiguousarray(in_proj_w[JQK:].T).astype(bf),
        "bqk": np.ascontiguousarray(
            in_proj_b[:JQK].reshape(JQK // 128, 128).T),
        "bvrow": in_proj_b[JQK:].reshape(1, E).astype(bf),
        "woutT": np.ascontiguousarray(np.asarray(out_proj_w, f).T).astype(bf),
        "bout": np.ascontiguousarray(
            np.asarray(out_proj_b, f).reshape(E // 128, 128).T),
        "wcT": np.ascontiguousarray(np.asarray(c_proj_w, f).T).astype(bf),
        "bc": np.ascontiguousarray(
            np.asarray(c_proj_b, f).reshape(E // 128, 128).T),
        "mask01": np.where(np.arange(128)[None, :] >= np.arange(128)[:, None],
                           f(1.0), f(0.0)).astype(bf),
        "onesbf": np.ones((128, H), bf),
        "ind": np.ascontiguousarray(ind.reshape(16, (H // 2) * 128)).astype(
            bf),
    }
    return [{**shared, "xT": np.ascontiguousarray(x[b].T).astype(bf)}
            for b in range(B)]


def kernel(x, in_proj_w, in_proj_b, out_proj_w, out_proj_b, c_proj_w,
           c_proj_b):
    nc = _build()
    in_maps = _host_inputs(x, in_proj_w, in_proj_b, out_proj_w, out_proj_b,
                           c_proj_w, c_proj_b)
    res = run_bass_kernel_spmd(nc, in_maps, core_ids=list(range(B)),
                               trace=TRACE)
    _CACHE["last_result"] = res
    out = np.stack([res.results[b]["outT"].T for b in range(B)])
    return np.ascontiguousarray(out, dtype=np.float32)
